# revision 6
# baseline (speedup 1.0000x reference)
"""CountSketch (scatter variant) as a Bass/Tile kernel for 8 TRN2 NeuronCores.

  out[b, i_hash[j]] += s_hash[j] * x[b, j]
  x: [16384, 8192] f32, i_hash/s_hash: [8192], out: [16384, 2048] f32

Strategy (data-parallel over batch, per sharding hint):
  - Shard batch across 8 cores: each core handles B_CORE=2048 rows.
  - The scatter along the feature axis is a segment-sum over d_in. On TRN2
    the only line-rate "gather/reduce" engine is the PE (matmul reduces over
    the partition dim), so each core's shard is laid out transposed
    [d_in, B_CORE] with d_in on partitions.
  - d_in columns are sorted by hash bucket (host-side, O(d_in) on the tiny
    hash tensors), so the 2048 buckets split into 16 groups of 128 features,
    and each 128-row chunk of the sorted layout touches only 1-2 groups.
  - Per (chunk, group) pair: one [128,128] one-hot +-sign weight matrix W
    (W[j, f] = s_j if i_hash_j == 128g+f else 0), built on device from tiny
    tables; matmul accumulates psum[128 f, B_CORE] over the group's chunks.
    out_T[128g:128g+128, :] = psum when the group completes.
  - Host transposes each core's out_T back and concatenates.

fp32 end to end; PSUM accumulation is fp32.
"""

import numpy as np

import concourse.bass as bass
import concourse.mybir as mybir
import concourse.tile as tile
from concourse import bacc
from concourse.bass_utils import run_bass_kernel_spmd

D_IN = 8192
D_FEATURES = 2048
BATCH = 16384
N_CORES = 8
B_CORE = BATCH // N_CORES  # 2048
P = 128
N_GROUPS = D_FEATURES // P  # 16
N_CHUNKS = D_IN // P  # 64
QN = 512  # fp32 matmul moving-operand free-dim limit (one PSUM bank)
CHUNKS_PER_LOAD = 4  # 4 chunks -> 4MB per DMA

# If True, device gathers sorted rows from the plainly-transposed shard via
# indirect DMA (index-driven, SWDGE). If False, the host lays out rows in
# sorted order and loads are contiguous HWDGE.
USE_INDIRECT = False

F32 = mybir.dt.float32
I32 = mybir.dt.int32


def _plan(i_hash: np.ndarray, s_hash: np.ndarray):
    """Host-side schedule from the tiny hash tensors."""
    i_hash = np.asarray(i_hash).astype(np.int64)
    s_hash = np.asarray(s_hash).astype(np.float32)
    perm = np.argsort(i_hash, kind="stable")
    b_sorted = i_hash[perm]
    g_sorted = b_sorted // P
    f_sorted = (b_sorted % P).astype(np.float32)
    s_sorted = s_hash[perm]

    pairs = []  # (chunk, group, f_local col f32[128], sign col f32[128])
    for c in range(N_CHUNKS):
        sl = slice(P * c, P * (c + 1))
        gs = g_sorted[sl]
        for g in sorted(set(gs.tolist())):
            m = gs == g
            pairs.append(
                (
                    c,
                    int(g),
                    np.where(m, f_sorted[sl], -1.0).astype(np.float32),
                    np.where(m, s_sorted[sl], 0.0).astype(np.float32),
                )
            )

    first_pair = {}
    last_pair = {}
    for idx, (c, g, _, _) in enumerate(pairs):
        first_pair.setdefault(g, idx)
        last_pair[g] = idx

    ftab = np.stack([p[2] for p in pairs], axis=1)  # [P, n_pairs]
    stab = np.stack([p[3] for p in pairs], axis=1)  # [P, n_pairs]
    meta = [(c, g) for (c, g, _, _) in pairs]
    return perm, meta, ftab, stab, first_pair, last_pair


def _build_nc(meta, first_pair, last_pair, n_pairs, b_core=B_CORE):
    nc = bacc.Bacc(None, target_bir_lowering=False)
    x_t = nc.dram_tensor("x_t", [D_IN, b_core], F32, kind="ExternalInput")
    ftab_d = nc.dram_tensor("ftab", [P, n_pairs], F32, kind="ExternalInput")
    stab_d = nc.dram_tensor("stab", [P, n_pairs], F32, kind="ExternalInput")
    iota_d = nc.dram_tensor("iota", [P, P], F32, kind="ExternalInput")
    if USE_INDIRECT:
        ridx_d = nc.dram_tensor("ridx", [P, N_CHUNKS], I32, kind="ExternalInput")
    out_t = nc.dram_tensor("out_t", [D_FEATURES, b_core], F32, kind="ExternalOutput")

    qn = min(QN, b_core)
    n_q = b_core // qn
    from contextlib import ExitStack

    with tile.TileContext(nc) as tc, ExitStack() as ctx:
        consts = ctx.enter_context(tc.tile_pool(name="consts", bufs=1))
        xpool = ctx.enter_context(tc.tile_pool(name="x", bufs=3))
        wpool = ctx.enter_context(tc.tile_pool(name="w", bufs=4))
        psum = ctx.enter_context(
            tc.tile_pool(name="psum", bufs=2, space=bass.MemorySpace.PSUM)
        )
        opool = ctx.enter_context(tc.tile_pool(name="o", bufs=2))

        ftab_sb = consts.tile([P, n_pairs], F32)
        nc.sync.dma_start(ftab_sb[:], ftab_d[:])
        stab_sb = consts.tile([P, n_pairs], F32)
        nc.sync.dma_start(stab_sb[:], stab_d[:])
        iota_sb = consts.tile([P, P], F32)
        nc.sync.dma_start(iota_sb[:], iota_d[:])
        if USE_INDIRECT:
            ridx_sb = consts.tile([P, N_CHUNKS], I32)
            nc.sync.dma_start(ridx_sb[:], ridx_d[:])

        x_tiles = {}  # load-block index -> tile

        def get_x_tile(c):
            blk = c // CHUNKS_PER_LOAD
            if blk not in x_tiles:
                xt = xpool.tile([P, CHUNKS_PER_LOAD, b_core], F32)
                if USE_INDIRECT:
                    for i in range(CHUNKS_PER_LOAD):
                        cc = blk * CHUNKS_PER_LOAD + i
                        nc.gpsimd.indirect_dma_start(
                            out=xt[:, i, :],
                            out_offset=None,
                            in_=x_t[:, :],
                            in_offset=bass.IndirectOffsetOnAxis(
                                ap=ridx_sb[:, cc : cc + 1], axis=0
                            ),
                        )
                else:
                    r0 = blk * CHUNKS_PER_LOAD * P
                    r1 = r0 + CHUNKS_PER_LOAD * P
                    nc.sync.dma_start(
                        xt[:],
                        x_t[r0:r1, :].rearrange("(c p) n -> p c n", p=P),
                    )
                x_tiles[blk] = xt
            return x_tiles[blk]

        cur_psum = None
        for pi, (c, g) in enumerate(meta):
            xt = get_x_tile(c)
            # Build W[j, f] = (iota_f == f_local_j) * s_j   ([P, P] one-hot)
            mask = wpool.tile([P, P], F32, tag="mask")
            nc.vector.tensor_tensor(
                out=mask[:],
                in0=ftab_sb[:, pi : pi + 1].to_broadcast([P, P]),
                in1=iota_sb[:],
                op=mybir.AluOpType.is_equal,
            )
            w = wpool.tile([P, P], F32, tag="w")
            nc.vector.tensor_scalar_mul(w[:], mask[:], stab_sb[:, pi : pi + 1])

            if pi == first_pair[g]:
                cur_psum = psum.tile([P, b_core], F32)
            for q in range(n_q):
                nc.tensor.matmul(
                    cur_psum[:, q * qn : (q + 1) * qn],
                    lhsT=w[:],
                    rhs=xt[:, c % CHUNKS_PER_LOAD, q * qn : (q + 1) * qn],
                    start=(pi == first_pair[g]),
                    stop=(pi == last_pair[g]),
                )
            if pi == last_pair[g]:
                ot = opool.tile([P, b_core], F32)
                nc.any.tensor_copy(ot[:], cur_psum[:])
                nc.sync.dma_start(out_t[g * P : (g + 1) * P, :], ot[:])

        # Groups with no hashed columns (possible in principle): zero-fill.
        for g in range(N_GROUPS):
            if g not in first_pair:
                ot = opool.tile([P, b_core], F32)
                nc.vector.memset(ot[:], 0.0)
                nc.sync.dma_start(out_t[g * P : (g + 1) * P, :], ot[:])

    nc.finalize()
    return nc


def _prepare(x, s_hash, i_hash):
    """Build the per-core input maps and the compiled Bass program."""
    perm, meta, ftab, stab, first_pair, last_pair = _plan(i_hash, s_hash)
    nc = _build_nc(meta, first_pair, last_pair, len(meta))

    iota = np.broadcast_to(np.arange(P, dtype=np.float32), (P, P)).copy()

    x = np.asarray(x, dtype=np.float32)
    in_maps = []
    for core in range(N_CORES):
        sh = x[core * B_CORE : (core + 1) * B_CORE]  # [B_CORE, D_IN]
        if USE_INDIRECT:
            x_dev = np.ascontiguousarray(sh.T)  # [D_IN, B_CORE]
        else:
            x_dev = np.ascontiguousarray(sh.T[perm])  # sorted rows
        m = {
            "x_t": x_dev,
            "ftab": np.ascontiguousarray(ftab),
            "stab": np.ascontiguousarray(stab),
            "iota": iota,
        }
        if USE_INDIRECT:
            m["ridx"] = np.ascontiguousarray(
                perm.astype(np.int32).reshape(N_CHUNKS, P).T
            )
        in_maps.append(m)
    return nc, in_maps


def _run(x, s_hash, i_hash, trace=False, **kw):
    nc, in_maps = _prepare(x, s_hash, i_hash)
    res = run_bass_kernel_spmd(nc, in_maps, list(range(N_CORES)), trace=trace, **kw)
    out = np.concatenate(
        [np.asarray(res.results[i]["out_t"]).T for i in range(N_CORES)], axis=0
    )
    return out.astype(np.float32), res


def kernel(x, s_hash, i_hash):
    out, _ = _run(x, s_hash, i_hash)
    return out


# revision 11
# speedup vs baseline: 30.6314x; 30.6314x over previous
"""CountSketch (scatter variant) as a Bass/Tile kernel for 8 TRN2 NeuronCores.

  out[b, i_hash[j]] += s_hash[j] * x[b, j]
  x: [16384, 8192] f32, i_hash/s_hash: [8192], out: [16384, 2048] f32

Strategy (data-parallel over batch, per sharding hint):
  - Shard batch across 8 cores: each core handles B_CORE=2048 rows.
  - The scatter along the feature axis is a segment-sum over d_in. On TRN2
    the only line-rate "gather/reduce" engine is the PE (matmul reduces over
    the partition dim), so each core's shard is laid out transposed
    [d_in, B_CORE] with d_in on partitions.
  - d_in columns are sorted by hash bucket (host-side, O(d_in) on the tiny
    hash tensors), so the 2048 buckets split into 16 groups of 128 features,
    and each 128-row chunk of the sorted layout touches only 1-2 groups.
  - Per (chunk, group) pair: one [128,128] one-hot +-sign weight matrix W
    (W[j, f] = s_j if i_hash_j == 128g+f else 0), built on device from tiny
    tables; matmul accumulates psum[128 f, B_CORE] over the group's chunks.
    out_T[128g:128g+128, :] = psum when the group completes.
  - Host transposes each core's out_T back and concatenates.

fp32 end to end; PSUM accumulation is fp32.
"""

import numpy as np

import concourse.bass as bass
import concourse.mybir as mybir
import concourse.tile as tile
from concourse import bacc
from concourse.bass_utils import run_bass_kernel_spmd

D_IN = 8192
D_FEATURES = 2048
BATCH = 16384
N_CORES = 8
B_CORE = BATCH // N_CORES  # 2048
P = 128
N_GROUPS = D_FEATURES // P  # 16
N_CHUNKS = D_IN // P  # 64
QN = 512  # fp32 matmul moving-operand free-dim limit (one PSUM bank)
CHUNKS_PER_LOAD = 4  # 4 chunks -> 4MB per DMA

# If True, device gathers sorted rows from the plainly-transposed shard via
# indirect DMA (index-driven, SWDGE). If False, the host lays out rows in
# sorted order and loads are contiguous HWDGE.
USE_INDIRECT = False

F32 = mybir.dt.float32
I32 = mybir.dt.int32


def _plan(i_hash: np.ndarray, s_hash: np.ndarray):
    """Host-side schedule from the tiny hash tensors."""
    i_hash = np.asarray(i_hash).astype(np.int64)
    s_hash = np.asarray(s_hash).astype(np.float32)
    perm = np.argsort(i_hash, kind="stable")
    b_sorted = i_hash[perm]
    g_sorted = b_sorted // P
    f_sorted = (b_sorted % P).astype(np.float32)
    s_sorted = s_hash[perm]

    pairs = []  # (chunk, group, f_local col f32[128], sign col f32[128])
    for c in range(N_CHUNKS):
        sl = slice(P * c, P * (c + 1))
        gs = g_sorted[sl]
        for g in sorted(set(gs.tolist())):
            m = gs == g
            pairs.append(
                (
                    c,
                    int(g),
                    np.where(m, f_sorted[sl], -1.0).astype(np.float32),
                    np.where(m, s_sorted[sl], 0.0).astype(np.float32),
                )
            )

    first_pair = {}
    last_pair = {}
    for idx, (c, g, _, _) in enumerate(pairs):
        first_pair.setdefault(g, idx)
        last_pair[g] = idx

    ftab = np.stack([p[2] for p in pairs], axis=1)  # [P, n_pairs]
    stab = np.stack([p[3] for p in pairs], axis=1)  # [P, n_pairs]
    meta = [(c, g) for (c, g, _, _) in pairs]
    return perm, meta, ftab, stab, first_pair, last_pair


def _build_nc(meta, first_pair, last_pair, n_pairs, b_core=B_CORE, reps=1):
    nc = bacc.Bacc(None, target_bir_lowering=False)
    x_t = nc.dram_tensor("x_t", [D_IN, b_core], F32, kind="ExternalInput")
    ftab_d = nc.dram_tensor("ftab", [P, n_pairs], F32, kind="ExternalInput")
    stab_d = nc.dram_tensor("stab", [P, n_pairs], F32, kind="ExternalInput")
    iota_d = nc.dram_tensor("iota", [P, P], F32, kind="ExternalInput")
    if USE_INDIRECT:
        ridx_d = nc.dram_tensor("ridx", [P, N_CHUNKS], I32, kind="ExternalInput")
    out_t = nc.dram_tensor("out_t", [D_FEATURES, b_core], F32, kind="ExternalOutput")

    qn = min(QN, b_core)
    n_q = b_core // qn
    from contextlib import ExitStack

    with tile.TileContext(nc) as tc, ExitStack() as ctx:
        consts = ctx.enter_context(tc.tile_pool(name="consts", bufs=1))
        xpool = ctx.enter_context(tc.tile_pool(name="x", bufs=3))
        wpool = ctx.enter_context(tc.tile_pool(name="w", bufs=4))
        psum = ctx.enter_context(
            tc.tile_pool(name="psum", bufs=2, space=bass.MemorySpace.PSUM)
        )
        opool = ctx.enter_context(tc.tile_pool(name="o", bufs=2))

        ftab_sb = consts.tile([P, n_pairs], F32)
        nc.sync.dma_start(ftab_sb[:], ftab_d[:])
        stab_sb = consts.tile([P, n_pairs], F32)
        nc.sync.dma_start(stab_sb[:], stab_d[:])
        iota_sb = consts.tile([P, P], F32)
        nc.sync.dma_start(iota_sb[:], iota_d[:])
        if USE_INDIRECT:
            ridx_sb = consts.tile([P, N_CHUNKS], I32)
            nc.sync.dma_start(ridx_sb[:], ridx_d[:])

        x_tiles = {}  # load-block index -> tile

        def get_x_tile(c, rep=0):
            cblk = c // CHUNKS_PER_LOAD
            key = (rep, cblk)
            if key not in x_tiles:
                xt = xpool.tile([P, CHUNKS_PER_LOAD, b_core], F32)
                if USE_INDIRECT:
                    for i in range(CHUNKS_PER_LOAD):
                        cc = cblk * CHUNKS_PER_LOAD + i
                        nc.gpsimd.indirect_dma_start(
                            out=xt[:, i, :],
                            out_offset=None,
                            in_=x_t[:, :],
                            in_offset=bass.IndirectOffsetOnAxis(
                                ap=ridx_sb[:, cc : cc + 1], axis=0
                            ),
                        )
                else:
                    r0 = cblk * CHUNKS_PER_LOAD * P
                    r1 = r0 + CHUNKS_PER_LOAD * P
                    nc.sync.dma_start(
                        xt[:],
                        x_t[r0:r1, :].rearrange("(c p) n -> p c n", p=P),
                    )
                x_tiles[key] = xt
            return x_tiles[key]

        for rep in range(reps):
            cur_psum = None
            for pi, (c, g) in enumerate(meta):
                xt = get_x_tile(c, rep)
                # Build W[j, f] = (iota_f == f_local_j) * s_j   ([P, P] one-hot)
                mask = wpool.tile([P, P], F32, tag="mask")
                nc.vector.tensor_tensor(
                    out=mask[:],
                    in0=ftab_sb[:, pi : pi + 1].to_broadcast([P, P]),
                    in1=iota_sb[:],
                    op=mybir.AluOpType.is_equal,
                )
                w = wpool.tile([P, P], F32, tag="w")
                nc.vector.tensor_scalar_mul(w[:], mask[:], stab_sb[:, pi : pi + 1])

                if pi == first_pair[g]:
                    cur_psum = psum.tile([P, b_core], F32)
                for q in range(n_q):
                    nc.tensor.matmul(
                        cur_psum[:, q * qn : (q + 1) * qn],
                        lhsT=w[:],
                        rhs=xt[:, c % CHUNKS_PER_LOAD, q * qn : (q + 1) * qn],
                        start=(pi == first_pair[g]),
                        stop=(pi == last_pair[g]),
                    )
                if pi == last_pair[g]:
                    ot = opool.tile([P, b_core], F32)
                    nc.any.tensor_copy(ot[:], cur_psum[:])
                    nc.sync.dma_start(out_t[g * P : (g + 1) * P, :], ot[:])

            # Groups with no hashed columns (possible in principle): zero-fill.
            for g in range(N_GROUPS):
                if g not in first_pair:
                    ot = opool.tile([P, b_core], F32)
                    nc.vector.memset(ot[:], 0.0)
                    nc.sync.dma_start(out_t[g * P : (g + 1) * P, :], ot[:])

    nc.finalize()
    return nc


def _prepare(x, s_hash, i_hash):
    """Build the per-core input maps and the compiled Bass program."""
    perm, meta, ftab, stab, first_pair, last_pair = _plan(i_hash, s_hash)
    nc = _build_nc(meta, first_pair, last_pair, len(meta))

    iota = np.broadcast_to(np.arange(P, dtype=np.float32), (P, P)).copy()

    x = np.asarray(x, dtype=np.float32)
    in_maps = []
    for core in range(N_CORES):
        sh = x[core * B_CORE : (core + 1) * B_CORE]  # [B_CORE, D_IN]
        if USE_INDIRECT:
            x_dev = np.ascontiguousarray(sh.T)  # [D_IN, B_CORE]
        else:
            x_dev = np.ascontiguousarray(sh.T[perm])  # sorted rows
        m = {
            "x_t": x_dev,
            "ftab": np.ascontiguousarray(ftab),
            "stab": np.ascontiguousarray(stab),
            "iota": iota,
        }
        if USE_INDIRECT:
            m["ridx"] = np.ascontiguousarray(
                perm.astype(np.int32).reshape(N_CHUNKS, P).T
            )
        in_maps.append(m)
    return nc, in_maps


def _run(x, s_hash, i_hash, trace=False, **kw):
    nc, in_maps = _prepare(x, s_hash, i_hash)
    res = run_bass_kernel_spmd(nc, in_maps, list(range(N_CORES)), trace=trace, **kw)
    out = np.concatenate(
        [np.asarray(res.results[i]["out_t"]).T for i in range(N_CORES)], axis=0
    )
    return out.astype(np.float32), res


def kernel(x, s_hash, i_hash):
    out, _ = _run(x, s_hash, i_hash)
    return out


# revision 16
# speedup vs baseline: 40.7263x; 1.3296x over previous
"""CountSketch (scatter variant) as a Bass/Tile kernel for 8 TRN2 NeuronCores.

  out[b, i_hash[j]] += s_hash[j] * x[b, j]
  x: [16384, 8192] f32, i_hash/s_hash: [8192], out: [16384, 2048] f32

Strategy (data-parallel over batch, per sharding hint):
  - Shard batch across 8 cores: each core handles B_CORE=2048 rows.
  - The scatter along the feature axis is a segment-sum over d_in. On TRN2
    the only line-rate "gather/reduce" engine is the PE (matmul reduces over
    the partition dim), so each core's shard is laid out transposed
    [d_in, B_CORE] with d_in on partitions.
  - d_in columns are sorted by hash bucket (host-side, O(d_in) on the tiny
    hash tensors), so the 2048 buckets split into 16 groups of 128 features,
    and each 128-row chunk of the sorted layout touches only 1-2 groups.
  - Per (chunk, group) pair: one [128,128] one-hot +-sign weight matrix W
    (W[j, f] = s_j if i_hash_j == 128g+f else 0), built on device from tiny
    tables; matmul accumulates psum[128 f, B_CORE] over the group's chunks.
    out_T[128g:128g+128, :] = psum when the group completes.
  - Host transposes each core's out_T back and concatenates.

fp32 end to end; PSUM accumulation is fp32.
"""

import numpy as np

import concourse.bass as bass
import concourse.mybir as mybir
import concourse.tile as tile
from concourse import bacc
from concourse.bass_utils import run_bass_kernel_spmd

D_IN = 8192
D_FEATURES = 2048
BATCH = 16384
N_CORES = 8
B_CORE = BATCH // N_CORES  # 2048
P = 128
N_GROUPS = D_FEATURES // P  # 16
N_CHUNKS = D_IN // P  # 64
QN = 512  # fp32 matmul moving-operand free-dim limit (one PSUM bank)
CHUNKS_PER_LOAD = 4  # 4 chunks -> 4MB per DMA
X_BUFS = 3  # x-tile pool double/triple buffering
STORE_BATCH = 1  # feature groups per output DMA

# If True, device gathers sorted rows from the plainly-transposed shard via
# indirect DMA (index-driven, SWDGE). If False, the host lays out rows in
# sorted order and loads are contiguous HWDGE.
USE_INDIRECT = False

F32 = mybir.dt.float32
I32 = mybir.dt.int32


def _plan(i_hash: np.ndarray, s_hash: np.ndarray):
    """Host-side schedule from the tiny hash tensors."""
    i_hash = np.asarray(i_hash).astype(np.int64)
    s_hash = np.asarray(s_hash).astype(np.float32)
    perm = np.argsort(i_hash, kind="stable")
    b_sorted = i_hash[perm]
    g_sorted = b_sorted // P
    f_sorted = (b_sorted % P).astype(np.float32)
    s_sorted = s_hash[perm]

    pairs = []  # (chunk, group, f_local col f32[128], sign col f32[128])
    for c in range(N_CHUNKS):
        sl = slice(P * c, P * (c + 1))
        gs = g_sorted[sl]
        for g in sorted(set(gs.tolist())):
            m = gs == g
            pairs.append(
                (
                    c,
                    int(g),
                    np.where(m, f_sorted[sl], -1.0).astype(np.float32),
                    np.where(m, s_sorted[sl], 0.0).astype(np.float32),
                )
            )

    first_pair = {}
    last_pair = {}
    for idx, (c, g, _, _) in enumerate(pairs):
        first_pair.setdefault(g, idx)
        last_pair[g] = idx

    ftab = np.stack([p[2] for p in pairs], axis=1)  # [P, n_pairs]
    stab = np.stack([p[3] for p in pairs], axis=1)  # [P, n_pairs]
    meta = [(c, g) for (c, g, _, _) in pairs]
    return perm, meta, ftab, stab, first_pair, last_pair


def _build_nc(meta, first_pair, last_pair, n_pairs, b_core=B_CORE, reps=1):
    nc = bacc.Bacc(None, target_bir_lowering=False)
    x_t = nc.dram_tensor("x_t", [D_IN, b_core], F32, kind="ExternalInput")
    ftab_d = nc.dram_tensor("ftab", [P, n_pairs], F32, kind="ExternalInput")
    stab_d = nc.dram_tensor("stab", [P, n_pairs], F32, kind="ExternalInput")
    iota_d = nc.dram_tensor("iota", [P, P], F32, kind="ExternalInput")
    if USE_INDIRECT:
        ridx_d = nc.dram_tensor("ridx", [P, N_CHUNKS], I32, kind="ExternalInput")
    out_t = nc.dram_tensor("out_t", [D_FEATURES, b_core], F32, kind="ExternalOutput")

    qn = min(QN, b_core)
    n_q = b_core // qn
    from contextlib import ExitStack

    with tile.TileContext(nc) as tc, ExitStack() as ctx:
        consts = ctx.enter_context(tc.tile_pool(name="consts", bufs=1))
        xpool = ctx.enter_context(tc.tile_pool(name="x", bufs=X_BUFS))
        wpool = ctx.enter_context(tc.tile_pool(name="w", bufs=4))
        psum = ctx.enter_context(
            tc.tile_pool(name="psum", bufs=2, space=bass.MemorySpace.PSUM)
        )
        opool = ctx.enter_context(tc.tile_pool(name="o", bufs=2))

        ftab_sb = consts.tile([P, n_pairs], F32)
        nc.sync.dma_start(ftab_sb[:], ftab_d[:])
        stab_sb = consts.tile([P, n_pairs], F32)
        nc.sync.dma_start(stab_sb[:], stab_d[:])
        iota_sb = consts.tile([P, P], F32)
        nc.sync.dma_start(iota_sb[:], iota_d[:])
        if USE_INDIRECT:
            ridx_sb = consts.tile([P, N_CHUNKS], I32)
            nc.sync.dma_start(ridx_sb[:], ridx_d[:])

        x_tiles = {}  # load-block index -> tile

        def get_x_tile(c, rep=0):
            cblk = c // CHUNKS_PER_LOAD
            key = (rep, cblk)
            if key not in x_tiles:
                xt = xpool.tile([P, CHUNKS_PER_LOAD, b_core], F32)
                if USE_INDIRECT:
                    for i in range(CHUNKS_PER_LOAD):
                        cc = cblk * CHUNKS_PER_LOAD + i
                        nc.gpsimd.indirect_dma_start(
                            out=xt[:, i, :],
                            out_offset=None,
                            in_=x_t[:, :],
                            in_offset=bass.IndirectOffsetOnAxis(
                                ap=ridx_sb[:, cc : cc + 1], axis=0
                            ),
                        )
                else:
                    r0 = cblk * CHUNKS_PER_LOAD * P
                    r1 = r0 + CHUNKS_PER_LOAD * P
                    nc.sync.dma_start(
                        xt[:],
                        x_t[r0:r1, :].rearrange("(c p) n -> p c n", p=P),
                    )
                x_tiles[key] = xt
            return x_tiles[key]

        for rep in range(reps):
            cur_psum = None
            for pi, (c, g) in enumerate(meta):
                xt = get_x_tile(c, rep)
                # Build W[j, f] = (iota_f == f_local_j) * s_j   ([P, P] one-hot)
                mask = wpool.tile([P, P], F32, tag="mask")
                nc.vector.tensor_tensor(
                    out=mask[:],
                    in0=ftab_sb[:, pi : pi + 1].to_broadcast([P, P]),
                    in1=iota_sb[:],
                    op=mybir.AluOpType.is_equal,
                )
                w = wpool.tile([P, P], F32, tag="w")
                nc.vector.tensor_scalar_mul(w[:], mask[:], stab_sb[:, pi : pi + 1])

                if pi == first_pair[g]:
                    cur_psum = psum.tile([P, b_core], F32)
                for q in range(n_q):
                    nc.tensor.matmul(
                        cur_psum[:, q * qn : (q + 1) * qn],
                        lhsT=w[:],
                        rhs=xt[:, c % CHUNKS_PER_LOAD, q * qn : (q + 1) * qn],
                        start=(pi == first_pair[g]),
                        stop=(pi == last_pair[g]),
                    )
                if pi == last_pair[g]:
                    sb = STORE_BATCH
                    gb = g // sb  # store-batch index
                    if g % sb == 0:
                        cur_ot = opool.tile([P, sb, b_core], F32)
                    nc.any.tensor_copy(cur_ot[:, g % sb, :], cur_psum[:])
                    if g % sb == sb - 1:
                        # scalar (ACT) HWDGE ring: keeps stores off the sync
                        # ring so they can't head-of-line-block x loads
                        nc.scalar.dma_start(
                            out_t[gb * sb * P : (gb + 1) * sb * P, :].rearrange(
                                "(c p) n -> p c n", p=P
                            ),
                            cur_ot[:],
                        )

            # Groups with no hashed columns (possible in principle): zero-fill.
            for g in range(N_GROUPS):
                if g not in first_pair:
                    ot = opool.tile([P, b_core], F32, tag="zfill")
                    nc.vector.memset(ot[:], 0.0)
                    nc.scalar.dma_start(out_t[g * P : (g + 1) * P, :], ot[:])

    nc.finalize()
    return nc


def _prepare(x, s_hash, i_hash):
    """Build the per-core input maps and the compiled Bass program."""
    perm, meta, ftab, stab, first_pair, last_pair = _plan(i_hash, s_hash)
    nc = _build_nc(meta, first_pair, last_pair, len(meta))

    iota = np.broadcast_to(np.arange(P, dtype=np.float32), (P, P)).copy()

    x = np.asarray(x, dtype=np.float32)
    in_maps = []
    for core in range(N_CORES):
        sh = x[core * B_CORE : (core + 1) * B_CORE]  # [B_CORE, D_IN]
        if USE_INDIRECT:
            x_dev = np.ascontiguousarray(sh.T)  # [D_IN, B_CORE]
        else:
            x_dev = np.ascontiguousarray(sh.T[perm])  # sorted rows
        m = {
            "x_t": x_dev,
            "ftab": np.ascontiguousarray(ftab),
            "stab": np.ascontiguousarray(stab),
            "iota": iota,
        }
        if USE_INDIRECT:
            m["ridx"] = np.ascontiguousarray(
                perm.astype(np.int32).reshape(N_CHUNKS, P).T
            )
        in_maps.append(m)
    return nc, in_maps


def _run(x, s_hash, i_hash, trace=False, **kw):
    nc, in_maps = _prepare(x, s_hash, i_hash)
    res = run_bass_kernel_spmd(nc, in_maps, list(range(N_CORES)), trace=trace, **kw)
    out = np.concatenate(
        [np.asarray(res.results[i]["out_t"]).T for i in range(N_CORES)], axis=0
    )
    return out.astype(np.float32), res


def kernel(x, s_hash, i_hash):
    out, _ = _run(x, s_hash, i_hash)
    return out
